# revision 61
# baseline (speedup 1.0000x reference)
"""Chunked gated-linear-attention (GLA) kernel for Trainium2, 8 NeuronCores.

Math (per (b,h), per-head scalar decay lam):
    S_t = lam * S_{t-1} + k_t^T v_t ;  o_t = (q_t * SCALE) @ S_t

Block form, chunk C=128, state updated every chunk. Host pre-scales
    qs_i = q_i * SCALE * lam^(i-64)        (i = index within chunk)
so on-chip:
    W[j,i]  = (k_j . qs_i) * lam^(64-j) * [j<=i]     (mask is causal*lam^(64-j))
    O[i]    = sum_j W[j,i] V[j] + qs_i . S'          (S' = lam^65 * S)
    S'     <- lam^128 S' + sum_j lam^(192-j) k_j v_j

Sharding: B*H = 32 (b,h) units, 4 per core (head-parallel, no collectives).
DMA: all inputs packed per 2-chunk block as [128, X] with 2-4KB/partition
contiguous descriptors, streamed on the sync (SP) HWDGE ring in exact
first-need order; output fp16 stores per block on the scalar (ACT) ring.
K is loaded once; K^T comes from tensor-engine transposes. The O-stage
(WV + qS matmuls, output copies) is software-pipelined one chunk behind
the W/state stage so DVE mask/cast latency never stalls the tensor queue.
PE warm-up matmuls on a memset tile hold the HAM clock gate open during
the initial DMA fill.
"""

import math
from contextlib import ExitStack

import numpy as np

import concourse.bacc as bacc
import concourse.mybir as mybir
import concourse.tile as tile
from concourse.bass_utils import run_bass_kernel_spmd

B, T, H, D = 2, 2048, 16, 128
C = 128                  # chunk size along time
NCH = T // C             # 16 chunks
BC = 2                   # chunks per DMA block
NB = NCH // BC           # 8 blocks
NCORES = 8
U = (B * H) // NCORES    # 4 (b,h) units per core
SCALE = 0.08838834764831845
LAYER_IDX, NUM_LAYERS = 12, 32

F32 = mybir.dt.float32
F16 = mybir.dt.float16

# cst0: [ident | maskc]; cst1: [ckm | sdg]; cstb: [qdm]
C0_ID = 0
C0_MASK = C
C0_W = C + U * C
C1_CKM = 0
C1_SDG = U * C
C1_W = 2 * U * C
CB_W = U * C
NWARM = 48               # PE warm-up matmuls at body start

TRACE = False            # test.py sets True to capture an NTFF profile
LAST = {}


def _slopes(n):
    def p2(m):
        start = 2.0 ** (-(2.0 ** (-(math.log2(m) - 3))))
        return [start * start**i for i in range(m)]

    if math.log2(n).is_integer():
        return p2(n)
    cp = 2 ** math.floor(math.log2(n))
    return p2(cp) + _slopes(2 * cp)[0::2][: n - cp]


def _lambdas():
    s = -np.asarray(_slopes(H), dtype=np.float64) * (
        1.0 - LAYER_IDX / (NUM_LAYERS - 1) + 1e-5
    )
    return np.exp(s)


def _build_nc():
    nc = bacc.Bacc(trn_type="TRN2", debug=False, num_devices=NCORES)

    # qt[b, d, (cc, u, i)] : Q^T, per 2-chunk block
    qt = nc.dram_tensor("qt", [NB, 128, BC * U * C], F16, kind="ExternalInput")
    # kv[b, p, (cc, u, x, d)] : K|V natural layout, per 2-chunk block
    kv = nc.dram_tensor("kv", [NB, 128, BC * U * 2 * D], F16, kind="ExternalInput")
    # s0[dk, (u, dv)]
    s0 = nc.dram_tensor("s0", [128, U * D], F16, kind="ExternalInput")
    cst0 = nc.dram_tensor("cst0", [128, C0_W], F16, kind="ExternalInput")
    cst1 = nc.dram_tensor("cst1", [128, C1_W], F16, kind="ExternalInput")
    cstb = nc.dram_tensor("cstb", [128, CB_W], F16, kind="ExternalInput")
    # o[b, i, (cc, u, dv)] fp16
    o = nc.dram_tensor("o", [NB, 128, BC * U * D], F16, kind="ExternalOutput")

    with tile.TileContext(nc) as tc, ExitStack() as ctx:
        const = ctx.enter_context(tc.tile_pool(name="const", bufs=1))
        ld = ctx.enter_context(tc.tile_pool(name="ld", bufs=1))
        h16 = ctx.enter_context(tc.tile_pool(name="h16", bufs=3))
        outp = ctx.enter_context(tc.tile_pool(name="outp", bufs=3))
        state = ctx.enter_context(tc.tile_pool(name="state", bufs=4))
        psum = ctx.enter_context(tc.tile_pool(name="psum", bufs=2, space="PSUM"))

        # ALL loads on the SP (sync) ring in strict first-need order; the
        # ACT (scalar) ring carries only output stores.
        qtiles, kvtiles = [], []

        def load_block(bn, eng):
            kvb = ld.tile(
                [128, BC * U * 2 * D], F16, tag="kvb", bufs=NB, name=f"kvb{bn}"
            )
            eng.dma_start(kvb[:], kv[bn])
            qb = ld.tile(
                [128, BC * U * C], F16, tag="qb", bufs=NB, name=f"qb{bn}"
            )
            eng.dma_start(qb[:], qt[bn])
            qtiles.append(qb)
            kvtiles.append(kvb)

        load_block(0, nc.sync)
        cst0_sb = const.tile([128, C0_W], F16)
        nc.sync.dma_start(cst0_sb[:], cst0[:])
        cst1_sb = const.tile([128, C1_W], F16)
        nc.sync.dma_start(cst1_sb[:], cst1[:])
        s_cur = state.tile([128, U * D], F16, tag="ssb")
        nc.sync.dma_start(s_cur[:], s0[:])
        cstb_sb = const.tile([128, CB_W], F16)
        nc.sync.dma_start(cstb_sb[:], cstb[:])
        for b in range(1, NB):
            load_block(b, nc.sync)

        ident = cst0_sb[:, C0_ID:C0_MASK]
        mask_sb = cst0_sb[:, C0_MASK:C0_W]
        ckm_sb = cst1_sb[:, C1_CKM:C1_SDG]
        sdg_sb = cst1_sb[:, C1_SDG:C1_W]
        qdm_sb = cstb_sb[:]

        # PE warm-up: dependency-free matmuls on a memset tile keep the
        # HAM clock gate open while the first DMAs stream in
        zwu = const.tile([128, 128], F16)
        nc.vector.memset(zwu[:], 0.0)
        pwu = psum.tile([128, U * C], F32, tag="w", name="pwu")
        for _ in range(NWARM):
            nc.tensor.matmul(
                pwu[:, 0:128], lhsT=zwu[:], rhs=zwu[:], start=True, stop=True
            )

        def views(bn):
            qbv = qtiles[bn][:].rearrange("p (cc u i) -> p cc u i", cc=BC, u=U)
            kvv = kvtiles[bn][:].rearrange(
                "p (cc u x d) -> p cc u x d", cc=BC, u=U, x=2
            )
            return qbv, kvv

        blockres = {}
        obtiles = {}

        def pre_transposes(bn, half):
            _, kvv = views(bn)
            if half == 0:
                pkt = psum.tile(
                    [128, BC * U * C], F16, tag="kt", name=f"pkt{bn}"
                )
                blockres[bn] = {"pkt": pkt}
            pkt = blockres[bn]["pkt"]
            for u in range(U):
                nc.tensor.transpose(
                    pkt[:, (half * U + u) * C : (half * U + u + 1) * C],
                    kvv[:, half, u, 0, :],
                    ident,
                )

        def pre_finish(bn):
            _, kvv = views(bn)
            ktb = h16.tile([128, BC * U * C], F16, tag="ktb", bufs=2)
            nc.vector.tensor_copy(ktb[:], blockres[bn]["pkt"][:])
            kd = h16.tile([128, BC * U * D], F16, tag="kd", bufs=2)
            for cc in range(BC):
                nc.vector.tensor_tensor(
                    kd[:, cc * U * D : (cc + 1) * U * D].rearrange(
                        "p (u d) -> p u d", u=U
                    ),
                    kvv[:, cc, :, 0, :],
                    ckm_sb.rearrange("p (u d) -> p u d", u=U),
                    mybir.AluOpType.mult,
                )
            # block 0 on DVE (gpsimd's Q7 lib load lands late in the
            # prologue); steady-state blocks on gpsimd to offload DVE
            eng = nc.vector if bn == 0 else nc.gpsimd
            qdec = h16.tile([128, BC * U * C], F16, tag="qdec", bufs=2)
            for cc in range(BC):
                eng.tensor_tensor(
                    qdec[:, cc * U * C : (cc + 1) * U * C],
                    qtiles[bn][:, cc * U * C : (cc + 1) * U * C],
                    qdm_sb,
                    mybir.AluOpType.mult,
                )
            blockres[bn]["ktb"] = ktb
            blockres[bn]["kd"] = kd
            blockres[bn]["qdec"] = qdec

        potiles = {}

        def emit_ostage(pv):
            """O = Wm^T V + qdec^T S for a pending chunk; copy+store per block."""
            pb, pcc, pwm, psv = pv
            _, pkvv = views(pb)
            pqdec = blockres[pb]["qdec"]
            if pcc == 0:
                potiles[pb] = psum.tile(
                    [128, BC * U * D], F32, tag="o", bufs=1, name=f"po{pb}"
                )
            po = potiles[pb]
            for u in range(U):
                ds = slice((pcc * U + u) * D, (pcc * U + u + 1) * D)
                ss = slice(u * D, (u + 1) * D)
                nc.tensor.matmul(
                    po[:, ds],
                    lhsT=pwm[:, u * C : (u + 1) * C],
                    rhs=pkvv[:, pcc, u, 1, :],
                    start=True,
                    stop=False,
                )
                nc.tensor.matmul(
                    po[:, ds],
                    lhsT=pqdec[:, (pcc * U + u) * C : (pcc * U + u + 1) * C],
                    rhs=psv[:, ss],
                    start=False,
                    stop=True,
                )
            if pb == NB - 1:
                # last block: per-chunk half copies + half stores so the
                # final store isn't gated on a full-block copy
                ob = blockres[pb].setdefault(
                    "ob",
                    outp.tile([128, BC * U * D], F16, tag="ob", name=f"ob{pb}"),
                )
                hs = slice(pcc * U * D, (pcc + 1) * U * D)
                nc.scalar.copy(ob[:, hs], po[:, hs])
                nc.scalar.dma_start(o[pb, :, hs], ob[:, hs])
                if pcc == BC - 1:
                    del potiles[pb]
            elif pcc == BC - 1:
                ob = outp.tile(
                    [128, BC * U * D], F16, tag="ob", name=f"ob{pb}"
                )
                nc.scalar.copy(ob[:], po[:])
                nc.scalar.dma_start(o[pb], ob[:])
                del potiles[pb]

        # full preamble for block 0
        pre_transposes(0, 0)
        pre_transposes(0, 1)
        pre_finish(0)

        pending = None
        for b in range(NB):
            qbv, kvv = views(b)
            br = blockres[b]
            ktb, kd = br["ktb"], br["kd"]

            for cc in range(BC):
                # W = K^T Qs
                pw = psum.tile([128, U * C], F32, tag="w")
                for u in range(U):
                    nc.tensor.matmul(
                        pw[:, u * C : (u + 1) * C],
                        lhsT=ktb[:, (cc * U + u) * C : (cc * U + u + 1) * C],
                        rhs=qbv[:, cc, u, :],
                        start=True,
                        stop=True,
                    )

                # state update: S <- lam^128 S + kd^T V (skip for the final
                # chunk: its updated state is never read)
                last_chunk = b == NB - 1 and cc == BC - 1
                if not last_chunk:
                    ps = psum.tile([128, U * D], F32, tag="s")
                    for u in range(U):
                        ds = slice(u * D, (u + 1) * D)
                        nc.tensor.matmul(
                            ps[:, ds],
                            lhsT=sdg_sb[:, ds],
                            rhs=s_cur[:, ds],
                            start=True,
                            stop=False,
                        )
                        nc.tensor.matmul(
                            ps[:, ds],
                            lhsT=kd[:, (cc * U + u) * D : (cc * U + u + 1) * D],
                            rhs=kvv[:, cc, u, 1, :],
                            start=False,
                            stop=True,
                        )
                    # state cast on ACT, ahead of output copies in its queue
                    s_new = state.tile([128, U * D], F16, tag="ssb")
                    nc.scalar.copy(s_new[:], ps[:])
                else:
                    s_new = s_cur

                # mask W (causal * lam^(64-j))
                wm = h16.tile([128, U * C], F16, tag="wm")
                nc.vector.tensor_tensor(
                    wm[:], pw[:], mask_sb[:], mybir.AluOpType.mult
                )

                # O-stage of the PREVIOUS chunk (one-chunk software pipeline)
                if pending is not None:
                    emit_ostage(pending)

                # next-block preamble at block end
                if cc == BC - 1 and b + 1 < NB:
                    pre_transposes(b + 1, 0)
                    pre_transposes(b + 1, 1)
                    pre_finish(b + 1)

                pending = (b, cc, wm, s_cur)
                s_cur = s_new

            if b - 1 in blockres:
                del blockres[b - 1]

        emit_ostage(pending)

    nc.compile()
    return nc


_NC_CACHE = []


def _get_nc():
    if not _NC_CACHE:
        _NC_CACHE.append(_build_nc())
    return _NC_CACHE[0]


def _core_consts(core):
    lam = _lambdas()
    i_idx = np.arange(C).astype(np.float64)
    c0v = np.zeros((128, C0_W), np.float16)
    c1v = np.zeros((128, C1_W), np.float16)
    cbv = np.zeros((128, CB_W), np.float16)
    eye = np.eye(128, dtype=np.float64)
    c0v[:, C0_ID : C0_ID + C] = np.eye(128, dtype=np.float16)
    for u in range(U):
        h = (U * core + u) % H
        l = lam[h]
        m = np.where(
            i_idx[None, :] >= i_idx[:, None],
            SCALE * l ** (i_idx[None, :] - i_idx[:, None]),
            0.0,
        )
        c0v[:, C0_MASK + u * C : C0_MASK + (u + 1) * C] = m.astype(np.float16)
        cq = (SCALE * l ** (i_idx + 1)).astype(np.float16)
        ck = (l ** (127.0 - i_idx)).astype(np.float16)
        cbv[:, u * C : (u + 1) * C] = np.tile(cq, (128, 1))
        c1v[:, C1_CKM + u * C : C1_CKM + (u + 1) * C] = np.repeat(
            ck[:, None], C, axis=1
        )
        c1v[:, C1_SDG + u * C : C1_SDG + (u + 1) * C] = (l**C * eye).astype(
            np.float16
        )
    return c0v, c1v, cbv


def kernel(query_states, key_states, value_states, initial_state):
    q16 = np.asarray(query_states).astype(np.float16)
    k16 = np.asarray(key_states).astype(np.float16)
    v16 = np.asarray(value_states).astype(np.float16)
    # [B,T,H,D] -> [B*H, T, D]
    q16 = np.transpose(q16, (0, 2, 1, 3)).reshape(B * H, T, D)
    k16 = np.transpose(k16, (0, 2, 1, 3)).reshape(B * H, T, D)
    v16 = np.transpose(v16, (0, 2, 1, 3)).reshape(B * H, T, D)
    s016 = np.asarray(initial_state).astype(np.float16).reshape(B * H, D, D)

    nc = _get_nc()
    in_maps = []
    for core in range(NCORES):
        lo = U * core
        c0v, c1v, cbv = _core_consts(core)
        qs = q16[lo : lo + U]  # [U, T, D]
        ks = k16[lo : lo + U]
        vs = v16[lo : lo + U]
        # qt[b, d, (cc,u,i)]
        qb = qs.reshape(U, NB, BC, C, D).transpose(1, 4, 2, 0, 3)
        qb = np.ascontiguousarray(qb.reshape(NB, 128, BC * U * C))
        # kv[b, p, (cc,u,x,d)]
        kvb = np.stack(
            [ks.reshape(U, NB, BC, C, D), vs.reshape(U, NB, BC, C, D)], axis=4
        )  # [U, NB, BC, C, 2, D]
        kvb = kvb.transpose(1, 3, 2, 0, 4, 5)  # [NB, C, BC, U, 2, D]
        kvb = np.ascontiguousarray(kvb.reshape(NB, 128, BC * U * 2 * D))
        s0b = np.ascontiguousarray(
            s016[lo : lo + U].transpose(1, 0, 2).reshape(128, U * D)
        )
        in_maps.append(
            {
                "qt": qb,
                "kv": kvb,
                "s0": s0b,
                "cst0": c0v,
                "cst1": c1v,
                "cstb": cbv,
            }
        )

    res = run_bass_kernel_spmd(
        nc, in_maps, core_ids=list(range(NCORES)), trace=TRACE
    )
    if TRACE:
        LAST["exec_time_ns"] = res.exec_time_ns
        LAST["mean_exec_time_ns"] = res.mean_exec_time_ns
        LAST["trace"] = (
            res.instructions_and_trace[1] if res.instructions_and_trace else None
        )

    out = np.empty((B * H, T, D), np.float32)
    for core in range(NCORES):
        ob = res.results[core]["o"].reshape(NB, C, BC, U, D)
        # -> [U, NB, BC, C, D] -> [U, T, D]
        out[U * core : U * core + U] = (
            ob.transpose(3, 0, 2, 1, 4).reshape(U, T, D).astype(np.float32)
        )
    return np.ascontiguousarray(
        np.transpose(out.reshape(B, H, T, D), (0, 2, 1, 3))
    )


# revision 64
# speedup vs baseline: 1.0481x; 1.0481x over previous
"""Chunked gated-linear-attention (GLA) kernel for Trainium2, 8 NeuronCores.

Math (per (b,h), per-head scalar decay lam):
    S_t = lam * S_{t-1} + k_t^T v_t ;  o_t = (q_t * SCALE) @ S_t

Block form, chunk C=128, state updated every chunk:
    W[j,i]  = (k_j . q_i) * SCALE*lam^(i-j) * [j<=i]
    O[i]    = sum_j W[j,i] V[j] + (q_i * SCALE*lam^(i+1)) . S
    S      <- lam^128 S + sum_j lam^(127-j) k_j v_j

Sharding: B*H = 32 (b,h) units, 4 per core (head-parallel, no collectives).
DMA: all inputs packed per 2-chunk block as [128, X] with 2-4KB/partition
contiguous descriptors, streamed on the sync (SP) HWDGE ring in exact
first-need order; output fp16 stores per block on the scalar (ACT) ring.
K is loaded once; K^T comes from tensor-engine transposes. The O-stage
(WV + qS matmuls, output copies) is software-pipelined one chunk behind
the W/state stage so DVE mask/cast latency never stalls the tensor queue.
PE warm-up matmuls on a memset tile hold the HAM clock gate open during
the initial DMA fill.
"""

import math
from contextlib import ExitStack

import numpy as np

import concourse.bacc as bacc
import concourse.mybir as mybir
import concourse.tile as tile
from concourse.bass_utils import run_bass_kernel_spmd

B, T, H, D = 2, 2048, 16, 128
C = 128                  # chunk size along time
NCH = T // C             # 16 chunks
BC = 2                   # chunks per DMA block
NB = NCH // BC           # 8 blocks
NCORES = 8
U = (B * H) // NCORES    # 4 (b,h) units per core
SCALE = 0.08838834764831845
LAYER_IDX, NUM_LAYERS = 12, 32

F32 = mybir.dt.float32
F16 = mybir.dt.float16

# cst0: [ident | maskc]; cst1: [ckm (x BC) | sdg]; cstb: [qdm (x BC)]
C0_ID = 0
C0_MASK = C
C0_W = C + U * C
C1_CKM = 0
C1_SDG = BC * U * C
C1_W = BC * U * C + U * C
CB_W = BC * U * C
NWARM = 48               # PE warm-up matmuls at body start

TRACE = False            # test.py sets True to capture an NTFF profile
LAST = {}


def _slopes(n):
    def p2(m):
        start = 2.0 ** (-(2.0 ** (-(math.log2(m) - 3))))
        return [start * start**i for i in range(m)]

    if math.log2(n).is_integer():
        return p2(n)
    cp = 2 ** math.floor(math.log2(n))
    return p2(cp) + _slopes(2 * cp)[0::2][: n - cp]


def _lambdas():
    s = -np.asarray(_slopes(H), dtype=np.float64) * (
        1.0 - LAYER_IDX / (NUM_LAYERS - 1) + 1e-5
    )
    return np.exp(s)


def _build_nc():
    nc = bacc.Bacc(trn_type="TRN2", debug=False, num_devices=NCORES)

    # qt[b, d, (cc, u, i)] : Q^T, per 2-chunk block
    qt = nc.dram_tensor("qt", [NB, 128, BC * U * C], F16, kind="ExternalInput")
    # kv[b, p, (cc, u, x, d)] : K|V natural layout, per 2-chunk block
    kv = nc.dram_tensor("kv", [NB, 128, BC * U * 2 * D], F16, kind="ExternalInput")
    # s0[dk, (u, dv)]
    s0 = nc.dram_tensor("s0", [128, U * D], F16, kind="ExternalInput")
    cst0 = nc.dram_tensor("cst0", [128, C0_W], F16, kind="ExternalInput")
    cst1 = nc.dram_tensor("cst1", [128, C1_W], F16, kind="ExternalInput")
    cstb = nc.dram_tensor("cstb", [128, CB_W], F16, kind="ExternalInput")
    # o[b, i, (cc, u, dv)] fp16
    o = nc.dram_tensor("o", [NB, 128, BC * U * D], F16, kind="ExternalOutput")

    with tile.TileContext(nc) as tc, ExitStack() as ctx:
        const = ctx.enter_context(tc.tile_pool(name="const", bufs=1))
        ld = ctx.enter_context(tc.tile_pool(name="ld", bufs=1))
        h16 = ctx.enter_context(tc.tile_pool(name="h16", bufs=3))
        outp = ctx.enter_context(tc.tile_pool(name="outp", bufs=3))
        state = ctx.enter_context(tc.tile_pool(name="state", bufs=4))
        psum = ctx.enter_context(tc.tile_pool(name="psum", bufs=2, space="PSUM"))

        # ALL loads on the SP (sync) ring in strict first-need order; the
        # ACT (scalar) ring carries only output stores.
        qtiles, kvtiles = [], []

        def load_block(bn, eng):
            kvb = ld.tile(
                [128, BC * U * 2 * D], F16, tag="kvb", bufs=NB, name=f"kvb{bn}"
            )
            eng.dma_start(kvb[:], kv[bn])
            qb = ld.tile(
                [128, BC * U * C], F16, tag="qb", bufs=NB, name=f"qb{bn}"
            )
            eng.dma_start(qb[:], qt[bn])
            qtiles.append(qb)
            kvtiles.append(kvb)

        load_block(0, nc.sync)
        cst0_sb = const.tile([128, C0_W], F16)
        nc.sync.dma_start(cst0_sb[:], cst0[:])
        cst1_sb = const.tile([128, C1_W], F16)
        nc.sync.dma_start(cst1_sb[:], cst1[:])
        s_cur = state.tile([128, U * D], F16, tag="ssb")
        nc.sync.dma_start(s_cur[:], s0[:])
        cstb_sb = const.tile([128, CB_W], F16)
        nc.sync.dma_start(cstb_sb[:], cstb[:])
        for b in range(1, NB):
            load_block(b, nc.sync)

        ident = cst0_sb[:, C0_ID:C0_MASK]
        mask_sb = cst0_sb[:, C0_MASK:C0_W]
        ckm_sb = cst1_sb[:, C1_CKM:C1_SDG]
        sdg_sb = cst1_sb[:, C1_SDG:C1_W]
        qdm_sb = cstb_sb[:]

        # PE warm-up: dependency-free matmuls on a memset tile keep the
        # HAM clock gate open while the first DMAs stream in
        zwu = const.tile([128, 128], F16)
        nc.vector.memset(zwu[:], 0.0)
        pwu = psum.tile([128, U * C], F32, tag="w", name="pwu")
        for _ in range(NWARM):
            nc.tensor.matmul(
                pwu[:, 0:128], lhsT=zwu[:], rhs=zwu[:], start=True, stop=True
            )

        def views(bn):
            qbv = qtiles[bn][:].rearrange("p (cc u i) -> p cc u i", cc=BC, u=U)
            kvv = kvtiles[bn][:].rearrange(
                "p (cc u x d) -> p cc u x d", cc=BC, u=U, x=2
            )
            return qbv, kvv

        blockres = {}

        def pre_transposes(bn, half):
            _, kvv = views(bn)
            if half == 0:
                pkt = psum.tile(
                    [128, BC * U * C], F16, tag="kt", name=f"pkt{bn}"
                )
                blockres[bn] = {"pkt": pkt}
            pkt = blockres[bn]["pkt"]
            for u in range(U):
                nc.tensor.transpose(
                    pkt[:, (half * U + u) * C : (half * U + u + 1) * C],
                    kvv[:, half, u, 0, :],
                    ident,
                )

        def pre_finish(bn):
            _, kvv = views(bn)
            ktb = h16.tile([128, BC * U * C], F16, tag="ktb", bufs=2)
            nc.vector.tensor_copy(ktb[:], blockres[bn]["pkt"][:])
            kd = h16.tile([128, BC * U * D], F16, tag="kd", bufs=2)
            nc.vector.tensor_tensor(
                kd[:].rearrange("p (cc u d) -> p cc u d", cc=BC, u=U),
                kvv[:, :, :, 0, :],
                ckm_sb.rearrange("p (cc u d) -> p cc u d", cc=BC, u=U),
                mybir.AluOpType.mult,
            )
            # block 0 on DVE (gpsimd's Q7 lib load lands late in the
            # prologue); steady-state blocks on gpsimd to offload DVE
            eng = nc.vector if bn == 0 else nc.gpsimd
            qdec = h16.tile([128, BC * U * C], F16, tag="qdec", bufs=2)
            eng.tensor_tensor(
                qdec[:], qtiles[bn][:], qdm_sb, mybir.AluOpType.mult
            )
            blockres[bn]["ktb"] = ktb
            blockres[bn]["kd"] = kd
            blockres[bn]["qdec"] = qdec

        potiles = {}

        def emit_ostage(pv):
            """O = Wm^T V + qdec^T S for a pending chunk; copy+store per block."""
            pb, pcc, pwm, psv = pv
            _, pkvv = views(pb)
            pqdec = blockres[pb]["qdec"]
            if pcc == 0:
                potiles[pb] = psum.tile(
                    [128, BC * U * D], F32, tag="o", bufs=1, name=f"po{pb}"
                )
            po = potiles[pb]
            for u in range(U):
                ds = slice((pcc * U + u) * D, (pcc * U + u + 1) * D)
                ss = slice(u * D, (u + 1) * D)
                nc.tensor.matmul(
                    po[:, ds],
                    lhsT=pwm[:, u * C : (u + 1) * C],
                    rhs=pkvv[:, pcc, u, 1, :],
                    start=True,
                    stop=False,
                )
                nc.tensor.matmul(
                    po[:, ds],
                    lhsT=pqdec[:, (pcc * U + u) * C : (pcc * U + u + 1) * C],
                    rhs=psv[:, ss],
                    start=False,
                    stop=True,
                )
            if pb == NB - 1:
                # last block: per-chunk half copies + half stores so the
                # final store isn't gated on a full-block copy
                ob = blockres[pb].setdefault(
                    "ob",
                    outp.tile([128, BC * U * D], F16, tag="ob", name=f"ob{pb}"),
                )
                hs = slice(pcc * U * D, (pcc + 1) * U * D)
                nc.scalar.copy(ob[:, hs], po[:, hs])
                nc.scalar.dma_start(o[pb, :, hs], ob[:, hs])
                if pcc == BC - 1:
                    del potiles[pb]
            elif pcc == BC - 1:
                ob = outp.tile(
                    [128, BC * U * D], F16, tag="ob", name=f"ob{pb}"
                )
                nc.scalar.copy(ob[:], po[:])
                nc.scalar.dma_start(o[pb], ob[:])
                del potiles[pb]

        # full preamble for block 0
        pre_transposes(0, 0)
        pre_transposes(0, 1)
        pre_finish(0)

        pending = None
        for b in range(NB):
            qbv, kvv = views(b)
            br = blockres[b]
            ktb, kd = br["ktb"], br["kd"]

            for cc in range(BC):
                # W = K^T Qs
                pw = psum.tile([128, U * C], F32, tag="w")
                for u in range(U):
                    nc.tensor.matmul(
                        pw[:, u * C : (u + 1) * C],
                        lhsT=ktb[:, (cc * U + u) * C : (cc * U + u + 1) * C],
                        rhs=qbv[:, cc, u, :],
                        start=True,
                        stop=True,
                    )

                # state update: S <- lam^128 S + kd^T V (skip for the final
                # chunk: its updated state is never read)
                last_chunk = b == NB - 1 and cc == BC - 1
                if not last_chunk:
                    ps = psum.tile([128, U * D], F32, tag="s")
                    for u in range(U):
                        ds = slice(u * D, (u + 1) * D)
                        nc.tensor.matmul(
                            ps[:, ds],
                            lhsT=sdg_sb[:, ds],
                            rhs=s_cur[:, ds],
                            start=True,
                            stop=False,
                        )
                        nc.tensor.matmul(
                            ps[:, ds],
                            lhsT=kd[:, (cc * U + u) * D : (cc * U + u + 1) * D],
                            rhs=kvv[:, cc, u, 1, :],
                            start=False,
                            stop=True,
                        )
                    # state cast on ACT, ahead of output copies in its queue
                    s_new = state.tile([128, U * D], F16, tag="ssb")
                    nc.scalar.copy(s_new[:], ps[:])
                else:
                    s_new = s_cur

                # mask W (causal * lam^(64-j))
                wm = h16.tile([128, U * C], F16, tag="wm")
                nc.vector.tensor_tensor(
                    wm[:], pw[:], mask_sb[:], mybir.AluOpType.mult
                )

                # O-stage of the PREVIOUS chunk (one-chunk software pipeline)
                if pending is not None:
                    emit_ostage(pending)

                # next-block preamble at block end
                if cc == BC - 1 and b + 1 < NB:
                    pre_transposes(b + 1, 0)
                    pre_transposes(b + 1, 1)
                    pre_finish(b + 1)

                pending = (b, cc, wm, s_cur)
                s_cur = s_new

            if b - 1 in blockres:
                del blockres[b - 1]

        emit_ostage(pending)

    nc.compile()
    return nc


_NC_CACHE = []


def _get_nc():
    if not _NC_CACHE:
        _NC_CACHE.append(_build_nc())
    return _NC_CACHE[0]


def _core_consts(core):
    lam = _lambdas()
    i_idx = np.arange(C).astype(np.float64)
    c0v = np.zeros((128, C0_W), np.float16)
    c1v = np.zeros((128, C1_W), np.float16)
    cbv = np.zeros((128, CB_W), np.float16)
    eye = np.eye(128, dtype=np.float64)
    c0v[:, C0_ID : C0_ID + C] = np.eye(128, dtype=np.float16)
    for u in range(U):
        h = (U * core + u) % H
        l = lam[h]
        m = np.where(
            i_idx[None, :] >= i_idx[:, None],
            SCALE * l ** (i_idx[None, :] - i_idx[:, None]),
            0.0,
        )
        c0v[:, C0_MASK + u * C : C0_MASK + (u + 1) * C] = m.astype(np.float16)
        cq = (SCALE * l ** (i_idx + 1)).astype(np.float16)
        ck = (l ** (127.0 - i_idx)).astype(np.float16)
        for cc in range(BC):
            off = (cc * U + u) * C
            cbv[:, off : off + C] = np.tile(cq, (128, 1))
            c1v[:, C1_CKM + off : C1_CKM + off + C] = np.repeat(
                ck[:, None], C, axis=1
            )
        c1v[:, C1_SDG + u * C : C1_SDG + (u + 1) * C] = (l**C * eye).astype(
            np.float16
        )
    return c0v, c1v, cbv


def kernel(query_states, key_states, value_states, initial_state):
    q16 = np.asarray(query_states).astype(np.float16)
    k16 = np.asarray(key_states).astype(np.float16)
    v16 = np.asarray(value_states).astype(np.float16)
    # [B,T,H,D] -> [B*H, T, D]
    q16 = np.transpose(q16, (0, 2, 1, 3)).reshape(B * H, T, D)
    k16 = np.transpose(k16, (0, 2, 1, 3)).reshape(B * H, T, D)
    v16 = np.transpose(v16, (0, 2, 1, 3)).reshape(B * H, T, D)
    s016 = np.asarray(initial_state).astype(np.float16).reshape(B * H, D, D)

    nc = _get_nc()
    in_maps = []
    for core in range(NCORES):
        lo = U * core
        c0v, c1v, cbv = _core_consts(core)
        qs = q16[lo : lo + U]  # [U, T, D]
        ks = k16[lo : lo + U]
        vs = v16[lo : lo + U]
        # qt[b, d, (cc,u,i)]
        qb = qs.reshape(U, NB, BC, C, D).transpose(1, 4, 2, 0, 3)
        qb = np.ascontiguousarray(qb.reshape(NB, 128, BC * U * C))
        # kv[b, p, (cc,u,x,d)]
        kvb = np.stack(
            [ks.reshape(U, NB, BC, C, D), vs.reshape(U, NB, BC, C, D)], axis=4
        )  # [U, NB, BC, C, 2, D]
        kvb = kvb.transpose(1, 3, 2, 0, 4, 5)  # [NB, C, BC, U, 2, D]
        kvb = np.ascontiguousarray(kvb.reshape(NB, 128, BC * U * 2 * D))
        s0b = np.ascontiguousarray(
            s016[lo : lo + U].transpose(1, 0, 2).reshape(128, U * D)
        )
        in_maps.append(
            {
                "qt": qb,
                "kv": kvb,
                "s0": s0b,
                "cst0": c0v,
                "cst1": c1v,
                "cstb": cbv,
            }
        )

    res = run_bass_kernel_spmd(
        nc, in_maps, core_ids=list(range(NCORES)), trace=TRACE
    )
    if TRACE:
        LAST["exec_time_ns"] = res.exec_time_ns
        LAST["mean_exec_time_ns"] = res.mean_exec_time_ns
        LAST["trace"] = (
            res.instructions_and_trace[1] if res.instructions_and_trace else None
        )

    out = np.empty((B * H, T, D), np.float32)
    for core in range(NCORES):
        ob = res.results[core]["o"].reshape(NB, C, BC, U, D)
        # -> [U, NB, BC, C, D] -> [U, T, D]
        out[U * core : U * core + U] = (
            ob.transpose(3, 0, 2, 1, 4).reshape(U, T, D).astype(np.float32)
        )
    return np.ascontiguousarray(
        np.transpose(out.reshape(B, H, T, D), (0, 2, 1, 3))
    )
